# revision 8
# baseline (speedup 1.0000x reference)
"""ArcFace head on 8 TRN2 NeuronCores.

Class-parallel sharding: core c owns classes [c*12500, (c+1)*12500).
Each core computes out[b, c_local] = S * (F_hat_b . W_hat_c) for its class
shard, and fixes up the target column of rows whose label it owns with the
ArcFace margin via an indirect-DMA scatter.

Host-side prep is layout/indexing only: shard + transpose + bf16-cast of the
weight, a gather of weight rows by label (wg), and flat scatter indices. All
FLOPs (norms, matmul, margin trig) run on device.
"""

import math
import os

import numpy as np

B = 512
D = 512
C = 100000
NCORES = 8
CS = C // NCORES  # 12500 classes per core

M_MARGIN = 0.5
S_SCALE = 64.0
TH = math.cos(math.pi - M_MARGIN)
MM_ = math.sin(math.pi - M_MARGIN) * M_MARGIN
EPS = 1e-12

P = 128
NB = B // P            # 4 b-chunks
NK = D // P            # 4 k-chunks
GW = 512               # c-group width (psum free dim)
NG = (CS + GW - 1) // GW          # 25 groups, last = 212 wide
SGG = 5                # groups per super-group (DMA/square granularity)
NSG = (NG + SGG - 1) // SGG       # 5 super-groups

OOB = 2 ** 28          # scatter index sentinel for unowned rows

_CACHE = {}


def _group_w(g):
    return min(GW, CS - g * GW)


def _sg_bounds(sg):
    lo = sg * SGG * GW
    hi = min(CS, (sg + 1) * SGG * GW)
    return lo, hi


def _build_nc():
    import concourse.tile as tile
    from concourse import bacc, mybir
    import concourse.bass as bass
    from concourse.masks import make_identity

    dt = mybir.dt
    Alu = mybir.AluOpType
    Act = mybir.ActivationFunctionType

    nc = bacc.Bacc("TRN2", target_bir_lowering=False, debug=False,
                   enable_asserts=False, num_devices=NCORES)

    wt = nc.dram_tensor("wt", [D, CS], dt.bfloat16, kind="ExternalInput").ap()
    feat = nc.dram_tensor("feat", [B, D], dt.float32, kind="ExternalInput").ap()
    wg = nc.dram_tensor("wg", [B, D], dt.float32, kind="ExternalInput").ap()
    sidx = nc.dram_tensor("sidx", [P, NB], dt.int32, kind="ExternalInput").ap()
    out = nc.dram_tensor("out", [B, CS], dt.float32, kind="ExternalOutput").ap()
    out_flat = out.rearrange("b c -> (b c)")[:, None]

    LN_S = math.log(S_SCALE)

    with tile.TileContext(nc) as tc:
        with (
            tc.tile_pool(name="const", bufs=1) as constp,
            tc.tile_pool(name="ph0", bufs=2) as ph0p,
            tc.tile_pool(name="wtp", bufs=8) as wtp,
            tc.tile_pool(name="sqp", bufs=8) as sqp,
            tc.tile_pool(name="invp", bufs=3) as invp,
            tc.tile_pool(name="outp", bufs=6) as outp,
            tc.tile_pool(name="ps_o", bufs=4, space="PSUM") as ps_o,
            tc.tile_pool(name="ps_n", bufs=2, space="PSUM") as ps_n,
            tc.tile_pool(name="ps_t", bufs=2, space="PSUM") as ps_t,
        ):
            # ---- constants ----
            ident = constp.tile([P, P], dt.bfloat16, tag="ident")
            make_identity(nc, ident[:])
            ones_sq = constp.tile([P, P], dt.bfloat16, tag="ones_sq")
            nc.vector.memset(ones_sq[:], 1.0)
            sidx_t = constp.tile([P, NB], dt.int32, tag="sidx_t")
            nc.sync.dma_start(sidx_t[:], sidx[:])
            lnS_t = constp.tile([P, 1], dt.float32, tag="lnS_t")
            nc.vector.memset(lnS_t[:], LN_S)

            # ---- phase 0: feature + gathered-weight normalization ----
            fn = []     # [128, 512] bf16, S-scaled normalized features
            val = []    # [128, 1] f32 margin-adjusted target logits
            for j in range(NB):
                f_t = ph0p.tile([P, D], dt.float32, tag=f"f{j}")
                nc.sync.dma_start(f_t[:], feat[j * P:(j + 1) * P, :])
                wg_t = ph0p.tile([P, D], dt.float32, tag=f"wg{j}")
                nc.sync.dma_start(wg_t[:], wg[j * P:(j + 1) * P, :])

                scr = ph0p.tile([P, D], dt.float32, tag="scr", bufs=2)
                fss = ph0p.tile([P, 1], dt.float32, tag=f"fss{j}")
                nc.vector.tensor_mul(scr[:], f_t[:], f_t[:])
                nc.vector.tensor_reduce(fss[:], scr[:],
                                        axis=mybir.AxisListType.X, op=Alu.add)
                # invf = S / sqrt(fss + EPS) = exp(-0.5*ln(fss+EPS) + ln S)
                lf = ph0p.tile([P, 1], dt.float32, tag=f"lf{j}")
                nc.scalar.activation(lf[:], fss[:], Act.Ln, bias=0.0, scale=1.0)
                invf = ph0p.tile([P, 1], dt.float32, tag=f"invf{j}")
                nc.scalar.activation(invf[:], lf[:], Act.Exp, bias=lnS_t[:, 0:1], scale=-0.5)
                fn_t = ph0p.tile([P, D], dt.bfloat16, tag=f"fn{j}")
                nc.vector.tensor_scalar_mul(fn_t[:], f_t[:], invf[:, 0:1])
                fn.append(fn_t)

                scr2 = ph0p.tile([P, D], dt.float32, tag="scr", bufs=2)
                wss = ph0p.tile([P, 1], dt.float32, tag=f"wss{j}")
                nc.vector.tensor_mul(scr2[:], wg_t[:], wg_t[:])
                nc.vector.tensor_reduce(wss[:], scr2[:],
                                        axis=mybir.AxisListType.X, op=Alu.add)
                lw = ph0p.tile([P, 1], dt.float32, tag=f"lw{j}")
                nc.scalar.activation(lw[:], wss[:], Act.Ln, bias=0.0, scale=1.0)
                invwg = ph0p.tile([P, 1], dt.float32, tag=f"invwg{j}")
                nc.scalar.activation(invwg[:], lw[:], Act.Exp, bias=0.0, scale=-0.5)
                wgn_t = ph0p.tile([P, D], dt.bfloat16, tag=f"wgn{j}")
                nc.vector.tensor_scalar_mul(wgn_t[:], wg_t[:], invwg[:, 0:1])

                # dot = S * tgt_cos
                scr3 = ph0p.tile([P, D], dt.float32, tag="scr", bufs=2)
                dot = ph0p.tile([P, 1], dt.float32, tag=f"dot{j}")
                nc.vector.tensor_mul(scr3[:], fn_t[:], wgn_t[:])
                nc.vector.tensor_reduce(dot[:], scr3[:],
                                        axis=mybir.AxisListType.X, op=Alu.add)

                # ---- margin on [128, 1] ----
                t = ph0p.tile([P, 1], dt.float32, tag=f"t{j}")
                nc.vector.tensor_scalar_mul(t[:], dot[:], 1.0 / S_SCALE)
                nc.vector.tensor_scalar_min(t[:], t[:], 1.0)
                nc.vector.tensor_scalar_max(t[:], t[:], -1.0)
                om = ph0p.tile([P, 1], dt.float32, tag=f"om{j}")
                nc.vector.tensor_mul(om[:], t[:], t[:])
                nc.vector.tensor_scalar(
                    out=om[:], in0=om[:], scalar1=-1.0, scalar2=1.0,
                    op0=Alu.mult, op1=Alu.add)
                lom = ph0p.tile([P, 1], dt.float32, tag=f"lom{j}")
                nc.scalar.activation(lom[:], om[:], Act.Ln, bias=0.0, scale=1.0)
                r = ph0p.tile([P, 1], dt.float32, tag=f"r{j}")
                nc.scalar.activation(r[:], lom[:], Act.Exp, bias=0.0, scale=0.5)
                a1 = ph0p.tile([P, 1], dt.float32, tag=f"a1{j}")
                nc.vector.tensor_scalar_mul(a1[:], t[:], math.cos(M_MARGIN))
                a2 = ph0p.tile([P, 1], dt.float32, tag=f"a2{j}")
                nc.vector.tensor_scalar_mul(a2[:], r[:], math.sin(M_MARGIN))
                adjA = ph0p.tile([P, 1], dt.float32, tag=f"adjA{j}")
                nc.vector.tensor_tensor(out=adjA[:], in0=a1[:], in1=a2[:],
                                        op=Alu.subtract)
                mask = ph0p.tile([P, 1], dt.int8, tag=f"mask{j}")
                nc.vector.tensor_scalar(
                    out=mask[:], in0=t[:], scalar1=TH, scalar2=None, op0=Alu.is_gt)
                adj = ph0p.tile([P, 1], dt.float32, tag=f"adj{j}")
                nc.vector.tensor_scalar_sub(adj[:], t[:], MM_)  # on_false branch
                nc.vector.copy_predicated(adj[:], mask[:], adjA[:])
                val_t = ph0p.tile([P, 1], dt.float32, tag=f"val{j}")
                nc.vector.tensor_scalar_mul(val_t[:], adj[:], S_SCALE)
                val.append(val_t)

            # ---- transpose fn -> fnT (stationary for main matmul) ----
            fnT = []
            for k in range(NK):
                fnT_k = constp.tile([P, B], dt.bfloat16, tag=f"fnT{k}")
                fnT.append(fnT_k)
            for k in range(NK):
                for j in range(NB):
                    pt = ps_t.tile([P, P], dt.bfloat16, tag="pt")
                    nc.tensor.transpose(pt[:], fn[j][:, k * P:(k + 1) * P], ident[:])
                    nc.vector.tensor_copy(fnT[k][:, j * P:(j + 1) * P], pt[:])

            # ---- main loop over class groups ----
            wt_tiles = [None] * NK
            sq_tiles = [None] * NK
            sg_lo = 0
            for g in range(NG):
                sg, gi = divmod(g, SGG)
                gw = _group_w(g)
                if gi == 0:
                    sg_lo, sg_hi = _sg_bounds(sg)
                    sgw = sg_hi - sg_lo
                    for k in range(NK):
                        wt_t = wtp.tile([P, SGG * GW], dt.bfloat16, tag=f"wt{k}",
                                        bufs=2)
                        nc.sync.dma_start(
                            wt_t[:, :sgw],
                            wt[k * P:(k + 1) * P, sg_lo:sg_hi])
                        sq_t = sqp.tile([P, SGG * GW], dt.bfloat16, tag=f"sq{k}",
                                        bufs=2)
                        nc.vector.tensor_tensor(
                            out=sq_t[:, :sgw], in0=wt_t[:, :sgw],
                            in1=wt_t[:, :sgw], op=Alu.mult)
                        wt_tiles[k] = wt_t
                        sq_tiles[k] = sq_t
                lo = g * GW - sg_lo  # offset within super-group tile

                # class inv-norms, replicated across partitions by the
                # all-ones stationary (same N-bound matmul cost)
                pn = ps_n.tile([P, GW], dt.float32, tag="pn")
                for k in range(NK):
                    nc.tensor.matmul(
                        pn[:, :gw], ones_sq[:], sq_tiles[k][:, lo:lo + gw],
                        start=(k == 0), stop=(k == NK - 1))
                lnrm = invp.tile([P, GW], dt.float32, tag="lnrm")
                nc.scalar.activation(lnrm[:, :gw], pn[:, :gw], Act.Ln,
                                     bias=0.0, scale=1.0)
                invb = invp.tile([P, GW], dt.float32, tag="invb")
                nc.scalar.activation(invb[:, :gw], lnrm[:, :gw], Act.Exp,
                                     bias=0.0, scale=-0.5)

                for j in range(NB):
                    po = ps_o.tile([P, GW], dt.float32, tag="po")
                    for k in range(NK):
                        nc.tensor.matmul(
                            po[:, :gw], fnT[k][:, j * P:(j + 1) * P],
                            wt_tiles[k][:, lo:lo + gw],
                            start=(k == 0), stop=(k == NK - 1))
                    ot = outp.tile([P, GW], dt.float32, tag="ot")
                    nc.vector.tensor_tensor(
                        out=ot[:, :gw], in0=po[:, :gw], in1=invb[:, :gw],
                        op=Alu.mult)
                    nc.sync.dma_start(
                        out[j * P:(j + 1) * P, g * GW:g * GW + gw], ot[:, :gw])

            # ---- margin scatter (ordered after all out writes by WAW dep) ----
            for j in range(NB):
                nc.gpsimd.indirect_dma_start(
                    out=out_flat,
                    out_offset=bass.IndirectOffsetOnAxis(ap=sidx_t[:, j:j + 1],
                                                         axis=0),
                    in_=val[j][:, 0:1],
                    in_offset=None,
                    bounds_check=B * CS - 1,
                    oob_is_err=False,
                )

    nc.compile()
    return nc


def _get_nc():
    if "nc" not in _CACHE:
        _CACHE["nc"] = _build_nc()
    return _CACHE["nc"]


def _enable_trace_hook():
    import sys
    import types
    import contextlib
    try:
        import antenv.axon_hooks  # noqa: F401
        return
    except ImportError:
        pass
    import antenv
    mod = types.ModuleType("antenv.axon_hooks")
    holder = [None]
    mod.set_axon_ntff_profile_hook = lambda h: holder.__setitem__(0, h)
    mod.get_axon_ntff_profile_hook = lambda: holder[0]
    sys.modules["antenv.axon_hooks"] = mod
    antenv.axon_hooks = mod
    try:
        from trn_agent_boot.trn_boot import _ntff_profile_via_ctypes
        mod.set_axon_ntff_profile_hook(
            _ntff_profile_via_ctypes("/opt/axon/libaxon_pjrt.so"))
    except Exception:
        pass


LAST_EXEC_NS = None
LAST_RESULTS = None


def kernel(features, labels, weight):
    global LAST_EXEC_NS, LAST_RESULTS
    import ml_dtypes
    from concourse.bass_utils import run_bass_kernel_spmd

    features = np.ascontiguousarray(np.asarray(features), dtype=np.float32)
    weight = np.asarray(weight)
    labels = np.asarray(labels).astype(np.int64)

    trace = bool(int(os.environ.get("ARCFACE_TRACE", "0")))
    if trace:
        _enable_trace_hook()

    nc = _get_nc()

    wt_bf16 = weight.astype(ml_dtypes.bfloat16)
    wg_full = np.ascontiguousarray(weight[labels], dtype=np.float32)

    rows = np.arange(B, dtype=np.int64)
    in_maps = []
    for c in range(NCORES):
        c0 = c * CS
        wt_c = np.ascontiguousarray(wt_bf16[c0:c0 + CS].T)  # [D, CS] bf16
        lab_loc = labels - c0
        owned = (labels >= c0) & (labels < c0 + CS)
        flat = np.where(owned, rows * CS + lab_loc, OOB).astype(np.int32)
        sidx_c = np.ascontiguousarray(flat.reshape(NB, P).T)  # [128, 4]
        in_maps.append({
            "wt": wt_c,
            "feat": features,
            "wg": wg_full,
            "sidx": sidx_c,
        })

    res = run_bass_kernel_spmd(nc, in_maps, core_ids=list(range(NCORES)),
                               trace=trace)
    LAST_EXEC_NS = res.exec_time_ns
    LAST_RESULTS = res
    shards = [res.results[c]["out"] for c in range(NCORES)]
    return np.concatenate(shards, axis=1)


# revision 9
# speedup vs baseline: 1.1562x; 1.1562x over previous
"""ArcFace head on 8 TRN2 NeuronCores.

Class-parallel sharding: core c owns classes [c*12500, (c+1)*12500).
Each core computes out[b, c_local] = S * (F_hat_b . W_hat_c) for its class
shard, and fixes up the target column of rows whose label it owns with the
ArcFace margin via an indirect-DMA scatter.

Host-side prep is layout/indexing only: shard + transpose + bf16-cast of the
weight, a gather of weight rows by label (wg), and flat scatter indices. All
FLOPs (norms, matmul, margin trig) run on device.
"""

import math
import os

import numpy as np

B = 512
D = 512
C = 100000
NCORES = 8
CS = C // NCORES  # 12500 classes per core

M_MARGIN = 0.5
S_SCALE = 64.0
TH = math.cos(math.pi - M_MARGIN)
MM_ = math.sin(math.pi - M_MARGIN) * M_MARGIN
EPS = 1e-12

P = 128
NB = B // P            # 4 b-chunks
NK = D // P            # 4 k-chunks
GW = 512               # c-group width (psum free dim)
NG = (CS + GW - 1) // GW          # 25 groups, last = 212 wide
SGG = 5                # groups per super-group (DMA/square granularity)
NSG = (NG + SGG - 1) // SGG       # 5 super-groups

OOB = 2 ** 28          # scatter index sentinel for unowned rows

_CACHE = {}


def _group_w(g):
    return min(GW, CS - g * GW)


def _sg_bounds(sg):
    lo = sg * SGG * GW
    hi = min(CS, (sg + 1) * SGG * GW)
    return lo, hi


def _build_nc():
    import concourse.tile as tile
    from concourse import bacc, mybir
    import concourse.bass as bass
    from concourse.masks import make_identity

    dt = mybir.dt
    Alu = mybir.AluOpType
    Act = mybir.ActivationFunctionType

    nc = bacc.Bacc("TRN2", target_bir_lowering=False, debug=False,
                   enable_asserts=False, num_devices=NCORES)

    wt = nc.dram_tensor("wt", [D, CS], dt.bfloat16, kind="ExternalInput").ap()
    feat = nc.dram_tensor("feat", [B, D], dt.float32, kind="ExternalInput").ap()
    wg = nc.dram_tensor("wg", [B, D], dt.float32, kind="ExternalInput").ap()
    sidx = nc.dram_tensor("sidx", [P, NB], dt.int32, kind="ExternalInput").ap()
    out = nc.dram_tensor("out", [B, CS], dt.float32, kind="ExternalOutput").ap()
    out_flat = out.rearrange("b c -> (b c)")[:, None]

    LN_S = math.log(S_SCALE)

    with tile.TileContext(nc) as tc:
        with (
            tc.tile_pool(name="const", bufs=1) as constp,
            tc.tile_pool(name="ph0", bufs=2) as ph0p,
            tc.tile_pool(name="wtp", bufs=8) as wtp,
            tc.tile_pool(name="sqp", bufs=8) as sqp,
            tc.tile_pool(name="invp", bufs=3) as invp,
            tc.tile_pool(name="outp", bufs=6) as outp,
            tc.tile_pool(name="ps_o", bufs=4, space="PSUM") as ps_o,
            tc.tile_pool(name="ps_n", bufs=2, space="PSUM") as ps_n,
            tc.tile_pool(name="ps_t", bufs=2, space="PSUM") as ps_t,
        ):
            # ---- constants ----
            ident = constp.tile([P, P], dt.bfloat16, tag="ident")
            make_identity(nc, ident[:])
            ones_sq = constp.tile([P, P], dt.bfloat16, tag="ones_sq")
            nc.vector.memset(ones_sq[:], 1.0)
            sidx_t = constp.tile([P, NB], dt.int32, tag="sidx_t")
            nc.sync.dma_start(sidx_t[:], sidx[:])

            # ---- phase 0: feature + gathered-weight normalization ----
            fn = []     # [128, 512] bf16, S-scaled normalized features
            val = []    # [128, 1] f32 margin-adjusted target logits
            for j in range(NB):
                f_t = ph0p.tile([P, D], dt.float32, tag=f"f{j}")
                nc.sync.dma_start(f_t[:], feat[j * P:(j + 1) * P, :])
                wg_t = ph0p.tile([P, D], dt.float32, tag=f"wg{j}")
                nc.sync.dma_start(wg_t[:], wg[j * P:(j + 1) * P, :])

                scr = ph0p.tile([P, D], dt.float32, tag="scr", bufs=2)
                fss = ph0p.tile([P, 1], dt.float32, tag=f"fss{j}")
                nc.vector.tensor_mul(scr[:], f_t[:], f_t[:])
                nc.vector.tensor_reduce(fss[:], scr[:],
                                        axis=mybir.AxisListType.X, op=Alu.add)
                # invf = S / sqrt(fss) = absrsqrt(fss / S^2)
                invf = ph0p.tile([P, 1], dt.float32, tag=f"invf{j}")
                nc.scalar.activation(invf[:], fss[:], Act.Abs_reciprocal_sqrt,
                                     bias=0.0, scale=1.0 / (S_SCALE * S_SCALE))
                fn_t = ph0p.tile([P, D], dt.bfloat16, tag=f"fn{j}")
                nc.vector.tensor_scalar_mul(fn_t[:], f_t[:], invf[:, 0:1])
                fn.append(fn_t)

                scr2 = ph0p.tile([P, D], dt.float32, tag="scr", bufs=2)
                wss = ph0p.tile([P, 1], dt.float32, tag=f"wss{j}")
                nc.vector.tensor_mul(scr2[:], wg_t[:], wg_t[:])
                nc.vector.tensor_reduce(wss[:], scr2[:],
                                        axis=mybir.AxisListType.X, op=Alu.add)
                invwg = ph0p.tile([P, 1], dt.float32, tag=f"invwg{j}")
                nc.scalar.activation(invwg[:], wss[:], Act.Abs_reciprocal_sqrt,
                                     bias=0.0, scale=1.0)
                wgn_t = ph0p.tile([P, D], dt.bfloat16, tag=f"wgn{j}")
                nc.vector.tensor_scalar_mul(wgn_t[:], wg_t[:], invwg[:, 0:1])

                # dot = S * tgt_cos
                scr3 = ph0p.tile([P, D], dt.float32, tag="scr", bufs=2)
                dot = ph0p.tile([P, 1], dt.float32, tag=f"dot{j}")
                nc.vector.tensor_mul(scr3[:], fn_t[:], wgn_t[:])
                nc.vector.tensor_reduce(dot[:], scr3[:],
                                        axis=mybir.AxisListType.X, op=Alu.add)

                # ---- margin on [128, 1] ----
                t = ph0p.tile([P, 1], dt.float32, tag=f"t{j}")
                nc.vector.tensor_scalar_mul(t[:], dot[:], 1.0 / S_SCALE)
                nc.vector.tensor_scalar_min(t[:], t[:], 1.0)
                nc.vector.tensor_scalar_max(t[:], t[:], -1.0)
                om = ph0p.tile([P, 1], dt.float32, tag=f"om{j}")
                nc.vector.tensor_mul(om[:], t[:], t[:])
                nc.vector.tensor_scalar(
                    out=om[:], in0=om[:], scalar1=-1.0, scalar2=1.0,
                    op0=Alu.mult, op1=Alu.add)
                rs = ph0p.tile([P, 1], dt.float32, tag=f"rs{j}")
                nc.scalar.activation(rs[:], om[:], Act.Abs_reciprocal_sqrt,
                                     bias=0.0, scale=1.0)
                r = ph0p.tile([P, 1], dt.float32, tag=f"r{j}")
                nc.vector.tensor_mul(r[:], om[:], rs[:])
                a1 = ph0p.tile([P, 1], dt.float32, tag=f"a1{j}")
                nc.vector.tensor_scalar_mul(a1[:], t[:], math.cos(M_MARGIN))
                a2 = ph0p.tile([P, 1], dt.float32, tag=f"a2{j}")
                nc.vector.tensor_scalar_mul(a2[:], r[:], math.sin(M_MARGIN))
                adjA = ph0p.tile([P, 1], dt.float32, tag=f"adjA{j}")
                nc.vector.tensor_tensor(out=adjA[:], in0=a1[:], in1=a2[:],
                                        op=Alu.subtract)
                mask = ph0p.tile([P, 1], dt.int8, tag=f"mask{j}")
                nc.vector.tensor_scalar(
                    out=mask[:], in0=t[:], scalar1=TH, scalar2=None, op0=Alu.is_gt)
                adj = ph0p.tile([P, 1], dt.float32, tag=f"adj{j}")
                nc.vector.tensor_scalar_sub(adj[:], t[:], MM_)  # on_false branch
                nc.vector.copy_predicated(adj[:], mask[:], adjA[:])
                val_t = ph0p.tile([P, 1], dt.float32, tag=f"val{j}")
                nc.vector.tensor_scalar_mul(val_t[:], adj[:], S_SCALE)
                val.append(val_t)

            # ---- transpose fn -> fnT (stationary for main matmul) ----
            fnT = []
            for k in range(NK):
                fnT_k = constp.tile([P, B], dt.bfloat16, tag=f"fnT{k}")
                fnT.append(fnT_k)
            for k in range(NK):
                for j in range(NB):
                    pt = ps_t.tile([P, P], dt.bfloat16, tag="pt")
                    nc.tensor.transpose(pt[:], fn[j][:, k * P:(k + 1) * P], ident[:])
                    nc.vector.tensor_copy(fnT[k][:, j * P:(j + 1) * P], pt[:])

            # ---- main loop over class groups ----
            wt_tiles = [None] * NK
            sq_tiles = [None] * NK
            sg_lo = 0
            for g in range(NG):
                sg, gi = divmod(g, SGG)
                gw = _group_w(g)
                if gi == 0:
                    sg_lo, sg_hi = _sg_bounds(sg)
                    sgw = sg_hi - sg_lo
                    for k in range(NK):
                        wt_t = wtp.tile([P, SGG * GW], dt.bfloat16, tag=f"wt{k}",
                                        bufs=2)
                        nc.sync.dma_start(
                            wt_t[:, :sgw],
                            wt[k * P:(k + 1) * P, sg_lo:sg_hi])
                        sq_t = sqp.tile([P, SGG * GW], dt.bfloat16, tag=f"sq{k}",
                                        bufs=2)
                        nc.vector.tensor_tensor(
                            out=sq_t[:, :sgw], in0=wt_t[:, :sgw],
                            in1=wt_t[:, :sgw], op=Alu.mult)
                        wt_tiles[k] = wt_t
                        sq_tiles[k] = sq_t
                lo = g * GW - sg_lo  # offset within super-group tile

                # class inv-norms, replicated across partitions by the
                # all-ones stationary (same N-bound matmul cost)
                pn = ps_n.tile([P, GW], dt.float32, tag="pn")
                for k in range(NK):
                    nc.tensor.matmul(
                        pn[:, :gw], ones_sq[:], sq_tiles[k][:, lo:lo + gw],
                        start=(k == 0), stop=(k == NK - 1))
                invb = invp.tile([P, GW], dt.float32, tag="invb")
                nc.scalar.activation(invb[:, :gw], pn[:, :gw],
                                     Act.Abs_reciprocal_sqrt, bias=0.0, scale=1.0)

                for j in range(NB):
                    po = ps_o.tile([P, GW], dt.float32, tag="po")
                    for k in range(NK):
                        nc.tensor.matmul(
                            po[:, :gw], fnT[k][:, j * P:(j + 1) * P],
                            wt_tiles[k][:, lo:lo + gw],
                            start=(k == 0), stop=(k == NK - 1))
                    ot = outp.tile([P, GW], dt.float32, tag="ot")
                    nc.vector.tensor_tensor(
                        out=ot[:, :gw], in0=po[:, :gw], in1=invb[:, :gw],
                        op=Alu.mult)
                    nc.sync.dma_start(
                        out[j * P:(j + 1) * P, g * GW:g * GW + gw], ot[:, :gw])

            # ---- margin scatter (ordered after all out writes by WAW dep) ----
            for j in range(NB):
                nc.gpsimd.indirect_dma_start(
                    out=out_flat,
                    out_offset=bass.IndirectOffsetOnAxis(ap=sidx_t[:, j:j + 1],
                                                         axis=0),
                    in_=val[j][:, 0:1],
                    in_offset=None,
                    bounds_check=B * CS - 1,
                    oob_is_err=False,
                )

    nc.compile()
    return nc


def _get_nc():
    if "nc" not in _CACHE:
        _CACHE["nc"] = _build_nc()
    return _CACHE["nc"]


def _enable_trace_hook():
    import sys
    import types
    import contextlib
    try:
        import antenv.axon_hooks  # noqa: F401
        return
    except ImportError:
        pass
    import antenv
    mod = types.ModuleType("antenv.axon_hooks")
    holder = [None]
    mod.set_axon_ntff_profile_hook = lambda h: holder.__setitem__(0, h)
    mod.get_axon_ntff_profile_hook = lambda: holder[0]
    sys.modules["antenv.axon_hooks"] = mod
    antenv.axon_hooks = mod
    try:
        from trn_agent_boot.trn_boot import _ntff_profile_via_ctypes
        mod.set_axon_ntff_profile_hook(
            _ntff_profile_via_ctypes("/opt/axon/libaxon_pjrt.so"))
    except Exception:
        pass


LAST_EXEC_NS = None
LAST_RESULTS = None


def kernel(features, labels, weight):
    global LAST_EXEC_NS, LAST_RESULTS
    import ml_dtypes
    from concourse.bass_utils import run_bass_kernel_spmd

    features = np.ascontiguousarray(np.asarray(features), dtype=np.float32)
    weight = np.asarray(weight)
    labels = np.asarray(labels).astype(np.int64)

    trace = bool(int(os.environ.get("ARCFACE_TRACE", "0")))
    if trace:
        _enable_trace_hook()

    nc = _get_nc()

    wt_bf16 = weight.astype(ml_dtypes.bfloat16)
    wg_full = np.ascontiguousarray(weight[labels], dtype=np.float32)

    rows = np.arange(B, dtype=np.int64)
    in_maps = []
    for c in range(NCORES):
        c0 = c * CS
        wt_c = np.ascontiguousarray(wt_bf16[c0:c0 + CS].T)  # [D, CS] bf16
        lab_loc = labels - c0
        owned = (labels >= c0) & (labels < c0 + CS)
        flat = np.where(owned, rows * CS + lab_loc, OOB).astype(np.int32)
        sidx_c = np.ascontiguousarray(flat.reshape(NB, P).T)  # [128, 4]
        in_maps.append({
            "wt": wt_c,
            "feat": features,
            "wg": wg_full,
            "sidx": sidx_c,
        })

    res = run_bass_kernel_spmd(nc, in_maps, core_ids=list(range(NCORES)),
                               trace=trace)
    LAST_EXEC_NS = res.exec_time_ns
    LAST_RESULTS = res
    shards = [res.results[c]["out"] for c in range(NCORES)]
    return np.concatenate(shards, axis=1)
